# revision 15
# baseline (speedup 1.0000x reference)
"""Trainium2 Bass kernel for nn_Attention (B=2, N=2048, C=768, H=12, D=64).

Sharding: 8 cores = 2 batches x 4 head-groups (3 heads each).
Per core: full attention for its (batch, 3 heads) + row-sharded proj
partial output [2048, 768]; host sums the 4 partials per batch (+b_proj).

v3 (pipelined, bf16 inputs): host supplies x TRANSPOSED, partition-major
packed and cast to bf16 (xt [4 chunks, 128, 6, 512]) so no on-chip
transposes of x are needed, every input DMA is contiguous per partition,
and input DMA bytes are halved (HBM bandwidth is shared by all 8 cores, so
startup is DMA-gated).  QKV is computed per 512-token chunk; scores+exp
for (q-chunk, k-chunk) combos are emitted as soon as both chunks' qkT are
ready, so the ACT engine (exp is ~100us of work, the pacing engine) starts
~12us into the kernel.  Probs (pt) live in a 32-buffer SBUF pool; PV
accumulation for q-chunk qc and 128-row proj pieces of earlier q-chunks
are interleaved at fine grain with the scores of later q-chunks (the PE
queue is in-order, so long serialized stretches stall everything).

Scores are computed TRANSPOSED (sT[k, q] = kT.T @ qT) via tile_position
row pairs; exp on ACT with scale folded in, no max subtraction.  PV
matmuls are M=65 (64 out dims + ones-column) so row 64 of each PSUM
accumulator is the softmax denominator.  Normalization: DVE copy of the
denominator row to SBUF -> SBUF-to-SBUF DMA reshape to [128,4] -> cheap
DVE reciprocal (multi-lane) -> DMA back to a row -> K=1 ones-matmul
broadcast (partition 64) -> DVE mul.  Final proj chunk runs after the exp
stream ends, with its own 4-bank PSUM pool and PSUM->SBUF copies split
between ACT and DVE.  Matmul dtypes: bf16 qkv/scores/PV, f32r proj.
"""

import numpy as np
import ml_dtypes

import concourse.bass as bass
import concourse.mybir as mybir
from concourse import bacc, tile
from concourse.bass_utils import run_bass_kernel_spmd
from concourse.masks import make_identity

F32 = mybir.dt.float32
F32R = mybir.dt.float32r
BF16 = mybir.dt.bfloat16
AF = mybir.ActivationFunctionType
BF16NP = ml_dtypes.bfloat16

B, N, C = 2, 2048, 768
H, D = 12, 64
SCALE = D ** -0.5  # 0.125
NCORES = 8
HPC = 3            # heads per core
NK = N // 128      # 16 k-tiles
NCH = 4            # 512-token chunks
WM = 704           # packed qkv weight columns: 5*128 + 64
CT = C // 128      # 6 c-tiles
TMM = [128, 128, 128, 128, 128, 64]  # qkv weight tile widths


def build_program():
    nc = bacc.Bacc("TRN2", target_bir_lowering=False, debug=False,
                   num_devices=NCORES)
    xt_d = nc.dram_tensor("xt", [NCH, 128, CT, 512], BF16, kind="ExternalInput")
    w_d = [nc.dram_tensor(f"w{t}", [128, CT, TMM[t]], BF16,
                          kind="ExternalInput") for t in range(6)]
    bq_d = nc.dram_tensor("bq", [128, 6], F32, kind="ExternalInput")
    wp_d = nc.dram_tensor("wp", [64, HPC, C], F32R, kind="ExternalInput")
    y_d = nc.dram_tensor("y", [N, C], F32, kind="ExternalOutput")

    with tile.TileContext(nc) as tc:
        with (
            tc.tile_pool(name="const", bufs=1) as cpool,
            tc.tile_pool(name="glob", bufs=1) as gpool,
            tc.tile_pool(name="pt", bufs=32) as ptpool,
            tc.tile_pool(name="rc", bufs=3) as rcpool,
            tc.tile_pool(name="y", bufs=3) as ypool,
        ):
            # warm the exp table set before any real dependency exists
            dummy = cpool.tile([1, 8], F32)
            nc.gpsimd.memset(dummy[:], 0.0)
            nc.scalar.activation(dummy[:], dummy[:], AF.Exp)

            ident = cpool.tile([128, 128], F32)
            make_identity(nc, ident[:])
            ones_f = cpool.tile([65, 64], F32)
            nc.gpsimd.memset(ones_f[:], 1.0)
            ones_sb = cpool.tile([65, 64], F32R)
            nc.vector.tensor_copy(ones_sb[:], ones_f[:])
            bq_sb = cpool.tile([128, 6], F32)
            nc.sync.dma_start(out=bq_sb[:], in_=bq_d[:])

            qkT = [gpool.tile([128, N], BF16, tag=f"qkT{t}", name=f"qkT{t}")
                   for t in range(4)]
            v_n = [gpool.tile([128, NK, 128], BF16, tag=f"vn{h}", name=f"vn{h}")
                   for h in range(HPC)]
            for h in range(HPC):
                nc.gpsimd.memset(v_n[h][:, :, 64:65], 1.0)
            outT = [gpool.tile([64, N], F32R, tag=f"outT{h}", name=f"outT{h}")
                    for h in range(HPC)]

            pts_pair = {}   # (qc, ktile) -> [128,1024] bf16 (h0 lo, h1 hi)
            pts_h2 = {}     # (qc, kpair) -> [128,1024] bf16
            accs = {}
            rrows = {}
            wp_r = gpool.tile([64, HPC, C], F32R)

            def emit_scores(qc, kc, scpool):
                """Scores + exp for q-chunk qc vs k-chunk kc (6 tiles)."""
                qs = slice(qc * 512, (qc + 1) * 512)
                for jj in range(4):
                    k = kc * 4 + jj
                    ks = slice(k * 128, (k + 1) * 128)
                    sc = scpool.tile([128, 1024], F32, tag="scores", name="sc")
                    nc.tensor.matmul(sc[:, 0:512], qkT[1][0:64, ks],
                                     qkT[0][0:64, qs], start=True, stop=True)
                    nc.tensor.matmul(sc[:, 512:1024], qkT[1][64:128, ks],
                                     qkT[0][64:128, qs], start=True,
                                     stop=True, tile_position=(64, 0))
                    pt = ptpool.tile([128, 1024], BF16, tag="pt", name="pt")
                    nc.scalar.activation(pt[:], sc[:], AF.Exp, scale=SCALE)
                    pts_pair[(qc, k)] = pt
                for jj in range(2):
                    kp = kc * 2 + jj
                    ke = slice((2 * kp) * 128, (2 * kp + 1) * 128)
                    ko = slice((2 * kp + 1) * 128, (2 * kp + 2) * 128)
                    sc = scpool.tile([128, 1024], F32, tag="scores", name="sc")
                    nc.tensor.matmul(sc[:, 0:512], qkT[3][0:64, ke],
                                     qkT[2][0:64, qs], start=True, stop=True)
                    nc.tensor.matmul(sc[:, 512:1024], qkT[3][64:128, ko],
                                     qkT[2][64:128, qs], start=True,
                                     stop=True, tile_position=(64, 0))
                    pt = ptpool.tile([128, 1024], BF16, tag="pt", name="pt")
                    nc.scalar.activation(pt[:], sc[:], AF.Exp, scale=SCALE)
                    pts_h2[(qc, kp)] = pt

            def pv_chunk(qc, c, acpool):
                """PV accumulation for q-chunk qc over k-chunk c (12 mm)."""
                if c == 0:
                    accs[qc] = [acpool.tile([128, 512], F32, tag="acc",
                                            bufs=3, name=f"s{h}_{qc}")
                                for h in range(HPC)]
                s = accs[qc]
                for jj in range(4):
                    k = c * 4 + jj
                    st = (c == 0 and jj == 0)
                    sp = (c == NCH - 1 and jj == 3)
                    nc.tensor.matmul(s[0][0:65, :], v_n[0][:, k, 0:65],
                                     pts_pair[(qc, k)][:, 0:512],
                                     start=st, stop=sp)
                for jj in range(4):
                    k = c * 4 + jj
                    st = (c == 0 and jj == 0)
                    sp = (c == NCH - 1 and jj == 3)
                    nc.tensor.matmul(s[1][0:65, :], v_n[1][:, k, 0:65],
                                     pts_pair[(qc, k)][:, 512:1024],
                                     start=st, stop=sp)
                for jj in range(2):
                    kp = c * 2 + jj
                    st = (c == 0 and jj == 0)
                    sp = (c == NCH - 1 and jj == 1)
                    nc.tensor.matmul(s[2][0:65, :], v_n[2][:, 2 * kp, 0:65],
                                     pts_h2[(qc, kp)][:, 0:512],
                                     start=st, stop=False)
                    nc.tensor.matmul(s[2][0:65, :],
                                     v_n[2][:, 2 * kp + 1, 0:65],
                                     pts_h2[(qc, kp)][:, 512:1024],
                                     start=False, stop=sp)

            def norm_copy(qc):
                """Denominator row -> [128,4] layout -> fast reciprocal ->
                back to a row at partition 64 (for the bcast matmul)."""
                rrows[qc] = []
                for h in range(HPC):
                    r = rcpool.tile([65, 512], F32R, tag="r", name="r")
                    nc.vector.tensor_copy(r[64:65, :], accs[qc][h][64:65, :])
                    r4 = rcpool.tile([128, 4], F32R, tag="r4", name="r4")
                    eng = nc.sync if h % 2 == 0 else nc.gpsimd
                    eng.dma_start(out=r4[:], in_=r[64:65, :])
                    r4r = rcpool.tile([128, 4], F32R, tag="r4r", name="r4r")
                    with nc.allow_low_precision(reason="f32r recip"):
                        nc.vector.reciprocal(r4r[:], r4[:])
                    rr = rcpool.tile([65, 512], F32R, tag="rr", name="rr")
                    eng2 = nc.gpsimd if h % 2 == 0 else nc.sync
                    eng2.dma_start(out=rr[64:65, :], in_=r4r[:])
                    rrows[qc].append(rr)

            def norm_fin(qc, pjpool):
                qs = slice(qc * 512, (qc + 1) * 512)
                for h in range(HPC):
                    bcs = pjpool.tile([64, 512], F32, tag="proj", bufs=1,
                                      name="bcs")
                    nc.tensor.matmul(bcs[0:64, :], ones_sb[64:65, 0:64],
                                     rrows[qc][h][64:65, :], start=True,
                                     stop=True, tile_position=(64, 0))
                    bcs_sb = rcpool.tile([64, 512], F32, tag="bcs",
                                         name="bcs_sb")
                    nc.vector.tensor_copy(bcs_sb[:], bcs[0:64, :])
                    nc.vector.tensor_mul(outT[h][0:64, qs],
                                         accs[qc][h][0:64, :], bcs_sb[:])

            def proj_piece(qc, j, pjpool, use_act=False):
                """One 128-row proj tile: y[qc*512+j*128 : +128, :]."""
                qj = slice(qc * 512 + j * 128, qc * 512 + (j + 1) * 128)
                y_sb = ypool.tile([128, C], F32, tag="y", name="ysb")
                pj = pjpool.tile([128, 512], F32, tag="proj", name="pj")
                for h in range(HPC):
                    nc.tensor.matmul(pj[:, 0:512], outT[h][0:64, qj],
                                     wp_r[0:64, h, 0:512],
                                     start=(h == 0), stop=(h == HPC - 1))
                eng = nc.scalar if use_act else nc.vector
                if use_act:
                    eng.copy(y_sb[:, 0:512], pj[:, 0:512])
                else:
                    eng.tensor_copy(y_sb[:, 0:512], pj[:, 0:512])
                pj2 = pjpool.tile([128, 512], F32, tag="proj", name="pj2")
                for h in range(HPC):
                    nc.tensor.matmul(pj2[:, 0:256], outT[h][0:64, qj],
                                     wp_r[0:64, h, 512:768],
                                     start=(h == 0), stop=(h == HPC - 1))
                nc.vector.tensor_copy(y_sb[:, 512:768], pj2[:, 0:256])
                nc.sync.dma_start(out=y_d[qj, :], in_=y_sb[:])

            # ---------------- Region 1: weights, qkv chunks, prefill ----------
            with (
                tc.tile_pool(name="xts", bufs=3) as xpool,
                tc.tile_pool(name="wr", bufs=1) as wrpool,
                tc.tile_pool(name="vtc", bufs=2) as vpool,
                tc.tile_pool(name="qkvps", bufs=2, space="PSUM") as qkvps,
                tc.tile_pool(name="tpps", bufs=1, space="PSUM") as tpps,
                tc.tile_pool(name="scps1", bufs=2, space="PSUM") as scpool1,
            ):
                # input DMAs, contiguous per partition; order = priority
                xts = []
                for c in range(NCH):
                    xt = xpool.tile([128, CT, 512], BF16, tag="xts",
                                    name=f"xts{c}")
                    xts.append(xt)
                nc.sync.dma_start(out=xts[0][:], in_=xt_d[0])
                w_rt = [wrpool.tile([128, CT, TMM[t]], BF16, tag=f"w{t}",
                                    name=f"w{t}") for t in range(6)]
                for t in (1, 0, 3, 2, 4, 5):
                    nc.scalar.dma_start(out=w_rt[t][:], in_=w_d[t].ap())
                nc.gpsimd.dma_start(out=xts[1][:], in_=xt_d[1])
                nc.sync.dma_start(out=xts[2][:], in_=xt_d[2])
                nc.gpsimd.dma_start(out=xts[3][:], in_=xt_d[3])
                nc.scalar.dma_start(out=wp_r[:], in_=wp_d.ap())

                def qkv_chunk(c):
                    ns = slice(c * 512, (c + 1) * 512)
                    vT4c = vpool.tile([128, 512], F32, tag="vt4",
                                      name=f"vt4_{c}")
                    vT5c = vpool.tile([64, 512], F32, tag="vt5",
                                      name=f"vt5_{c}")
                    for t in (1, 0, 3, 2, 4, 5):
                        mm = TMM[t]
                        qps = qkvps.tile([128, 512], F32, tag="qkv",
                                         name=f"qps{t}_{c}")
                        for ct in range(CT):
                            nc.tensor.matmul(qps[0:mm, :], w_rt[t][:, ct, :],
                                             xts[c][:, ct, :], start=(ct == 0),
                                             stop=(ct == CT - 1))
                        bias = (bq_sb[:, t:t + 1] if mm == 128
                                else bq_sb[0:mm, t:t + 1])
                        if t < 4:
                            dst = qkT[t][:, ns]
                        elif t == 4:
                            dst = vT4c[:, :]
                        else:
                            dst = vT5c[0:64, :]
                        nc.vector.tensor_scalar(dst, qps[0:mm, :], bias, None,
                                                mybir.AluOpType.add)
                    # transpose v slices into v_n [keys, d]
                    vsrc = [(vT4c[0:64, :], ident[0:64, 0:64]),
                            (vT4c[64:128, :], ident[64:128, 64:128]),
                            (vT5c[0:64, :], ident[0:64, 0:64])]
                    for h in range(HPC):
                        srcv, idn = vsrc[h]
                        tp = tpps.tile([128, 256], F32, tag="tp")
                        for jj in range(4):
                            nc.tensor.transpose(tp[:, jj * 64:(jj + 1) * 64],
                                                srcv[:, jj * 128:(jj + 1) * 128],
                                                idn)
                        nc.vector.tensor_copy(
                            v_n[h][:, c * 4:(c + 1) * 4, 0:64],
                            tp[:].rearrange("p (j d) -> p j d", j=4))

                qkv_chunk(0)
                emit_scores(0, 0, scpool1)
                qkv_chunk(1)
                emit_scores(0, 1, scpool1)
                emit_scores(1, 0, scpool1)
                emit_scores(1, 1, scpool1)
                qkv_chunk(2)
                emit_scores(0, 2, scpool1)
                qkv_chunk(3)

            # ------------- Region 2: steady interleave of scores/PV/proj ------
            with (
                tc.tile_pool(name="scps2", bufs=2, space="PSUM") as scpool2,
                tc.tile_pool(name="acc", bufs=3, space="PSUM") as acpool,
                tc.tile_pool(name="pj", bufs=1, space="PSUM") as pjpool,
            ):
                emit_scores(0, 3, scpool2)
                pv_chunk(0, 0, acpool)
                emit_scores(1, 2, scpool2)
                pv_chunk(0, 1, acpool)
                emit_scores(1, 3, scpool2)
                pv_chunk(0, 2, acpool)
                emit_scores(2, 0, scpool2)
                pv_chunk(0, 3, acpool)
                norm_copy(0)
                emit_scores(2, 1, scpool2)
                norm_fin(0, pjpool)
                pv_chunk(1, 0, acpool)
                emit_scores(2, 2, scpool2)
                proj_piece(0, 0, pjpool)
                pv_chunk(1, 1, acpool)
                emit_scores(2, 3, scpool2)
                proj_piece(0, 1, pjpool)
                pv_chunk(1, 2, acpool)
                emit_scores(3, 0, scpool2)
                proj_piece(0, 2, pjpool)
                pv_chunk(1, 3, acpool)
                norm_copy(1)
                emit_scores(3, 1, scpool2)
                norm_fin(1, pjpool)
                proj_piece(0, 3, pjpool)
                pv_chunk(2, 0, acpool)
                emit_scores(3, 2, scpool2)
                proj_piece(1, 0, pjpool)
                pv_chunk(2, 1, acpool)
                emit_scores(3, 3, scpool2)
                proj_piece(1, 1, pjpool)
                pv_chunk(2, 2, acpool)
                proj_piece(1, 2, pjpool)
                pv_chunk(2, 3, acpool)
                norm_copy(2)
                proj_piece(1, 3, pjpool)
                norm_fin(2, pjpool)
                pv_chunk(3, 0, acpool)
                proj_piece(2, 0, pjpool)
                pv_chunk(3, 1, acpool)
                proj_piece(2, 1, pjpool)
                pv_chunk(3, 2, acpool)
                proj_piece(2, 2, pjpool)
                pv_chunk(3, 3, acpool)
                norm_copy(3)
                proj_piece(2, 3, pjpool)
                norm_fin(3, pjpool)

            # ------------- Region 3: final proj with deep pipelining ----------
            with tc.tile_pool(name="pjB", bufs=4, space="PSUM") as pjpoolB:
                for j in range(4):
                    proj_piece(3, j, pjpoolB, use_act=True)

    nc.compile()
    return nc


def make_in_maps(x, w_qkv, b_qkv, w_proj):
    """Per-core input dicts. Core c: batch c//4, heads 3*(c%4)+[0..2]."""
    x = np.asarray(x, np.float32)
    w_qkv = np.asarray(w_qkv, np.float32)
    b_qkv = np.asarray(b_qkv, np.float32)
    w_proj = np.asarray(w_proj, np.float32)
    q = lambda h: w_qkv[:, h * 64:(h + 1) * 64]
    k = lambda h: w_qkv[:, C + h * 64: C + (h + 1) * 64]
    v = lambda h: w_qkv[:, 2 * C + h * 64: 2 * C + (h + 1) * 64]
    qb = lambda h: b_qkv[h * 64:(h + 1) * 64]
    kb = lambda h: b_qkv[C + h * 64: C + (h + 1) * 64]
    vb = lambda h: b_qkv[2 * C + h * 64: 2 * C + (h + 1) * 64]
    # xt[c, p, t, n] = x[b][c*512+n, t*128+p], bf16
    xt = [np.ascontiguousarray(
        x[b].reshape(NCH, 512, CT, 128).transpose(0, 3, 2, 1)
    ).astype(BF16NP) for b in range(B)]
    in_maps = []
    for c in range(NCORES):
        b = c // 4
        h0 = 3 * (c % 4)
        h1, h2 = h0 + 1, h0 + 2
        w_pack = np.concatenate(
            [q(h0), q(h1), k(h0), k(h1), q(h2), q(h2), k(h2), k(h2),
             v(h0), v(h1), v(h2)], axis=1).astype(np.float32)
        bias = np.concatenate(
            [qb(h0), qb(h1), kb(h0), kb(h1), qb(h2), qb(h2), kb(h2), kb(h2),
             vb(h0), vb(h1), vb(h2), np.zeros(64, np.float32)])
        bq_pack = bias.reshape(6, 128).T.copy()  # [128, 6]
        wp_pack = np.stack(
            [w_proj[h * 64:(h + 1) * 64, :] for h in (h0, h1, h2)])
        im = {
            "xt": xt[b],
            "bq": np.ascontiguousarray(bq_pack),
            "wp": np.ascontiguousarray(wp_pack.transpose(1, 0, 2)),
        }
        off = 0
        for t in range(6):
            mm = TMM[t]
            blk = w_pack[:, off:off + mm]  # [768, mm]
            off += mm
            # w{t}[p, ct, m] = w_pack[ct*128+p, t_off+m]
            im[f"w{t}"] = np.ascontiguousarray(
                blk.reshape(CT, 128, mm).transpose(1, 0, 2)).astype(BF16NP)
        in_maps.append(im)
    return in_maps


_NC_CACHE = []


def _get_program():
    if not _NC_CACHE:
        _NC_CACHE.append(build_program())
    return _NC_CACHE[0]


def run(inputs, trace=False, **kw):
    nc = _get_program()
    in_maps = make_in_maps(inputs["x"], inputs["w_qkv"], inputs["b_qkv"],
                           inputs["w_proj"])
    res = run_bass_kernel_spmd(nc, in_maps, list(range(NCORES)), trace=trace, **kw)
    b_proj = np.asarray(inputs["b_proj"], np.float32)
    out = np.zeros((B, N, C), np.float32)
    for c in range(NCORES):
        out[c // 4] += res.results[c]["y"]
    out += b_proj[None, None, :]
    return out.astype(np.float32), res


def kernel(**inputs):
    out, _ = run(inputs)
    return out


# revision 18
# speedup vs baseline: 1.1987x; 1.1987x over previous
"""Trainium2 Bass kernel for nn_Attention (B=2, N=2048, C=768, H=12, D=64).

Sharding: 8 cores = 2 batches x 4 head-groups (3 heads each).
Per core: full attention for its (batch, 3 heads) + row-sharded proj
partial output [2048, 768]; host sums the 4 partials per batch (+b_proj).

v3 (pipelined, bf16 inputs): host supplies x TRANSPOSED, partition-major
packed and cast to bf16 (xt [4 chunks, 128, 6, 512]) so no on-chip
transposes of x are needed, every input DMA is contiguous per partition,
and input DMA bytes are halved (HBM bandwidth is shared by all 8 cores, so
startup is DMA-gated).  QKV is computed per 512-token chunk; scores+exp
for (q-chunk, k-chunk) combos are emitted as soon as both chunks' qkT are
ready, so the ACT engine (exp is ~100us of work, the pacing engine) starts
~12us into the kernel.  Probs (pt) live in a 32-buffer SBUF pool; PV
accumulation for q-chunk qc and 128-row proj pieces of earlier q-chunks
are interleaved at fine grain with the scores of later q-chunks (the PE
queue is in-order, so long serialized stretches stall everything).

Scores are computed TRANSPOSED (sT[k, q] = kT.T @ qT) via tile_position
row pairs; exp on ACT with scale folded in, no max subtraction.  PV
matmuls are M=65 (64 out dims + ones-column) so row 64 of each PSUM
accumulator is the softmax denominator.  Normalization: DVE copy of the
denominator row to SBUF -> SBUF-to-SBUF DMA reshape to [128,4] -> cheap
DVE reciprocal (multi-lane) -> DMA back to a row -> K=1 ones-matmul
broadcast (partition 64) -> DVE mul.  Final proj chunk runs after the exp
stream ends, with its own 4-bank PSUM pool and PSUM->SBUF copies split
between ACT and DVE.  Matmul dtypes: bf16 qkv/scores/PV, f32r proj.
"""

import numpy as np
import ml_dtypes

import concourse.bass as bass
import concourse.mybir as mybir
from concourse import bacc, tile
from concourse.bass_utils import run_bass_kernel_spmd
from concourse.masks import make_identity

F32 = mybir.dt.float32
F32R = mybir.dt.float32r
BF16 = mybir.dt.bfloat16
AF = mybir.ActivationFunctionType
BF16NP = ml_dtypes.bfloat16

B, N, C = 2, 2048, 768
H, D = 12, 64
SCALE = D ** -0.5  # 0.125
NCORES = 8
HPC = 3            # heads per core
NK = N // 128      # 16 k-tiles
NCH = 4            # 512-token chunks
WM = 704           # packed qkv weight columns: 5*128 + 64
CT = C // 128      # 6 c-tiles
TMM = [128, 128, 128, 128, 128, 64]  # qkv weight tile widths


def build_program():
    nc = bacc.Bacc("TRN2", target_bir_lowering=False, debug=False,
                   num_devices=NCORES)
    xt_d = nc.dram_tensor("xt", [NCH, 128, CT, 512], BF16, kind="ExternalInput")
    w_d = [nc.dram_tensor(f"w{t}", [128, CT, TMM[t]], BF16,
                          kind="ExternalInput") for t in range(6)]
    bq_d = nc.dram_tensor("bq", [128, 6], F32, kind="ExternalInput")
    wp_d = nc.dram_tensor("wp", [64, HPC, C], F32R, kind="ExternalInput")
    y_d = nc.dram_tensor("y", [N, C], F32, kind="ExternalOutput")

    with tile.TileContext(nc) as tc:
        with (
            tc.tile_pool(name="const", bufs=1) as cpool,
            tc.tile_pool(name="glob", bufs=1) as gpool,
            tc.tile_pool(name="pt", bufs=32) as ptpool,
            tc.tile_pool(name="rc", bufs=3) as rcpool,
            tc.tile_pool(name="y", bufs=3) as ypool,
        ):
            # warm the exp table set before any real dependency exists
            dummy = cpool.tile([1, 8], F32)
            nc.gpsimd.memset(dummy[:], 0.0)
            nc.scalar.activation(dummy[:], dummy[:], AF.Exp)

            ident = cpool.tile([128, 128], F32)
            make_identity(nc, ident[:])
            ones_f = cpool.tile([65, 64], F32)
            nc.gpsimd.memset(ones_f[:], 1.0)
            ones_sb = cpool.tile([65, 64], F32R)
            nc.vector.tensor_copy(ones_sb[:], ones_f[:])
            bq_sb = cpool.tile([128, 6], F32)
            nc.sync.dma_start(out=bq_sb[:], in_=bq_d[:])

            qkT = [gpool.tile([128, N], BF16, tag=f"qkT{t}", name=f"qkT{t}")
                   for t in range(4)]
            v_n = [gpool.tile([128, NK, 128], BF16, tag=f"vn{h}", name=f"vn{h}")
                   for h in range(HPC)]
            for h in range(HPC):
                nc.gpsimd.memset(v_n[h][:, :, 64:65], 1.0)
            outT = [gpool.tile([64, N], F32R, tag=f"outT{h}", name=f"outT{h}")
                    for h in range(HPC)]

            pts_pair = {}   # (qc, ktile) -> [128,1024] bf16 (h0 lo, h1 hi)
            pts_h2 = {}     # (qc, kpair) -> [128,1024] bf16
            accs = {}
            rrows = {}
            wp_r = gpool.tile([64, HPC, C], F32R)

            def emit_scores(qc, kc, scpool):
                """Scores + exp for q-chunk qc vs k-chunk kc (6 tiles)."""
                qs = slice(qc * 512, (qc + 1) * 512)
                for jj in range(4):
                    k = kc * 4 + jj
                    ks = slice(k * 128, (k + 1) * 128)
                    sc = scpool.tile([128, 1024], F32, tag="scores", name="sc")
                    nc.tensor.matmul(sc[:, 0:512], qkT[1][0:64, ks],
                                     qkT[0][0:64, qs], start=True, stop=True)
                    nc.tensor.matmul(sc[:, 512:1024], qkT[1][64:128, ks],
                                     qkT[0][64:128, qs], start=True,
                                     stop=True, tile_position=(64, 0))
                    pt = ptpool.tile([128, 1024], BF16, tag="pt", name="pt")
                    nc.scalar.activation(pt[:], sc[:], AF.Exp, scale=SCALE)
                    pts_pair[(qc, k)] = pt
                for jj in range(2):
                    kp = kc * 2 + jj
                    ke = slice((2 * kp) * 128, (2 * kp + 1) * 128)
                    ko = slice((2 * kp + 1) * 128, (2 * kp + 2) * 128)
                    sc = scpool.tile([128, 1024], F32, tag="scores", name="sc")
                    nc.tensor.matmul(sc[:, 0:512], qkT[3][0:64, ke],
                                     qkT[2][0:64, qs], start=True, stop=True)
                    nc.tensor.matmul(sc[:, 512:1024], qkT[3][64:128, ko],
                                     qkT[2][64:128, qs], start=True,
                                     stop=True, tile_position=(64, 0))
                    pt = ptpool.tile([128, 1024], BF16, tag="pt", name="pt")
                    nc.scalar.activation(pt[:], sc[:], AF.Exp, scale=SCALE)
                    pts_h2[(qc, kp)] = pt

            def pv_chunk(qc, c, acpool):
                """PV accumulation for q-chunk qc over k-chunk c (12 mm)."""
                if c == 0:
                    accs[qc] = [acpool.tile([128, 512], F32, tag="acc",
                                            bufs=3, name=f"s{h}_{qc}")
                                for h in range(HPC)]
                s = accs[qc]
                for jj in range(4):
                    k = c * 4 + jj
                    st = (c == 0 and jj == 0)
                    sp = (c == NCH - 1 and jj == 3)
                    nc.tensor.matmul(s[0][0:65, :], v_n[0][:, k, 0:65],
                                     pts_pair[(qc, k)][:, 0:512],
                                     start=st, stop=sp)
                for jj in range(4):
                    k = c * 4 + jj
                    st = (c == 0 and jj == 0)
                    sp = (c == NCH - 1 and jj == 3)
                    nc.tensor.matmul(s[1][0:65, :], v_n[1][:, k, 0:65],
                                     pts_pair[(qc, k)][:, 512:1024],
                                     start=st, stop=sp)
                for jj in range(2):
                    kp = c * 2 + jj
                    st = (c == 0 and jj == 0)
                    sp = (c == NCH - 1 and jj == 1)
                    nc.tensor.matmul(s[2][0:65, :], v_n[2][:, 2 * kp, 0:65],
                                     pts_h2[(qc, kp)][:, 0:512],
                                     start=st, stop=False)
                    nc.tensor.matmul(s[2][0:65, :],
                                     v_n[2][:, 2 * kp + 1, 0:65],
                                     pts_h2[(qc, kp)][:, 512:1024],
                                     start=False, stop=sp)

            def norm_copy(qc):
                """Denominator row -> [128,4] layout -> fast reciprocal ->
                back to a row at partition 64 (for the bcast matmul)."""
                rrows[qc] = []
                for h in range(HPC):
                    r = rcpool.tile([65, 512], F32R, tag="r", name="r")
                    nc.vector.tensor_copy(r[64:65, :], accs[qc][h][64:65, :])
                    r4 = rcpool.tile([128, 4], F32R, tag="r4", name="r4")
                    eng = nc.sync if h % 2 == 0 else nc.gpsimd
                    eng.dma_start(out=r4[:], in_=r[64:65, :])
                    r4r = rcpool.tile([128, 4], F32R, tag="r4r", name="r4r")
                    with nc.allow_low_precision(reason="f32r recip"):
                        nc.vector.reciprocal(r4r[:], r4[:])
                    rr = rcpool.tile([65, 512], F32R, tag="rr", name="rr")
                    eng2 = nc.gpsimd if h % 2 == 0 else nc.sync
                    eng2.dma_start(out=rr[64:65, :], in_=r4r[:])
                    rrows[qc].append(rr)

            def norm_fin(qc, pjpool):
                qs = slice(qc * 512, (qc + 1) * 512)
                for h in range(HPC):
                    bcs = pjpool.tile([64, 512], F32, tag="proj", bufs=1,
                                      name="bcs")
                    nc.tensor.matmul(bcs[0:64, :], ones_sb[64:65, 0:64],
                                     rrows[qc][h][64:65, :], start=True,
                                     stop=True, tile_position=(64, 0))
                    bcs_sb = rcpool.tile([64, 512], F32, tag="bcs",
                                         name="bcs_sb")
                    nc.vector.tensor_copy(bcs_sb[:], bcs[0:64, :])
                    nc.vector.tensor_mul(outT[h][0:64, qs],
                                         accs[qc][h][0:64, :], bcs_sb[:])

            def proj_piece(qc, j, pjpool, use_act=False):
                """One 128-row proj tile: y[qc*512+j*128 : +128, :].

                PSUM comes from the scores pool rotation (paced by exp, so
                the matmuls here never head-of-line-block the PE queue)."""
                qj = slice(qc * 512 + j * 128, qc * 512 + (j + 1) * 128)
                y_sb = ypool.tile([128, C], F32, tag="y", name="ysb")
                pj = pjpool.tile([128, 1024], F32, tag="scores", name="pj")
                for h in range(HPC):
                    nc.tensor.matmul(pj[:, 0:512], outT[h][0:64, qj],
                                     wp_r[0:64, h, 0:512],
                                     start=(h == 0), stop=(h == HPC - 1))
                for h in range(HPC):
                    nc.tensor.matmul(pj[:, 512:768], outT[h][0:64, qj],
                                     wp_r[0:64, h, 512:768],
                                     start=(h == 0), stop=(h == HPC - 1))
                if use_act:
                    nc.scalar.copy(y_sb[:, 0:512], pj[:, 0:512])
                else:
                    nc.vector.tensor_copy(y_sb[:, 0:512], pj[:, 0:512])
                nc.vector.tensor_copy(y_sb[:, 512:768], pj[:, 512:768])
                nc.sync.dma_start(out=y_d[qj, :], in_=y_sb[:])

            # ---------------- Region 1: weights, qkv chunks, prefill ----------
            with (
                tc.tile_pool(name="xts", bufs=3) as xpool,
                tc.tile_pool(name="wr", bufs=1) as wrpool,
                tc.tile_pool(name="vtc", bufs=2) as vpool,
                tc.tile_pool(name="qkvps", bufs=2, space="PSUM") as qkvps,
                tc.tile_pool(name="tpps", bufs=1, space="PSUM") as tpps,
                tc.tile_pool(name="scps1", bufs=2, space="PSUM") as scpool1,
            ):
                # input DMAs, contiguous per partition; order = priority
                xts = []
                for c in range(NCH):
                    xt = xpool.tile([128, CT, 512], BF16, tag="xts",
                                    name=f"xts{c}")
                    xts.append(xt)
                nc.sync.dma_start(out=xts[0][:, 0:3, :], in_=xt_d[0][:, 0:3, :])
                nc.gpsimd.dma_start(out=xts[0][:, 3:6, :],
                                    in_=xt_d[0][:, 3:6, :])
                w_rt = [wrpool.tile([128, CT, TMM[t]], BF16, tag=f"w{t}",
                                    name=f"w{t}") for t in range(6)]
                for t in (1, 0, 3, 2, 4, 5):
                    nc.scalar.dma_start(out=w_rt[t][:], in_=w_d[t].ap())
                nc.gpsimd.dma_start(out=xts[1][:], in_=xt_d[1])
                nc.sync.dma_start(out=xts[2][:], in_=xt_d[2])
                nc.gpsimd.dma_start(out=xts[3][:], in_=xt_d[3])
                nc.scalar.dma_start(out=wp_r[:], in_=wp_d.ap())

                def qkv_chunk(c):
                    ns = slice(c * 512, (c + 1) * 512)
                    vT4c = vpool.tile([128, 512], F32, tag="vt4",
                                      name=f"vt4_{c}")
                    vT5c = vpool.tile([64, 512], F32, tag="vt5",
                                      name=f"vt5_{c}")
                    for t in (1, 0, 3, 2, 4, 5):
                        mm = TMM[t]
                        qps = qkvps.tile([128, 512], F32, tag="qkv",
                                         name=f"qps{t}_{c}")
                        for ct in range(CT):
                            nc.tensor.matmul(qps[0:mm, :], w_rt[t][:, ct, :],
                                             xts[c][:, ct, :], start=(ct == 0),
                                             stop=(ct == CT - 1))
                        bias = (bq_sb[:, t:t + 1] if mm == 128
                                else bq_sb[0:mm, t:t + 1])
                        if t < 4:
                            dst = qkT[t][:, ns]
                        elif t == 4:
                            dst = vT4c[:, :]
                        else:
                            dst = vT5c[0:64, :]
                        nc.vector.tensor_scalar(dst, qps[0:mm, :], bias, None,
                                                mybir.AluOpType.add)
                    # transpose v slices into v_n [keys, d]
                    vsrc = [(vT4c[0:64, :], ident[0:64, 0:64]),
                            (vT4c[64:128, :], ident[64:128, 64:128]),
                            (vT5c[0:64, :], ident[0:64, 0:64])]
                    for h in range(HPC):
                        srcv, idn = vsrc[h]
                        tp = tpps.tile([128, 256], F32, tag="tp")
                        for jj in range(4):
                            nc.tensor.transpose(tp[:, jj * 64:(jj + 1) * 64],
                                                srcv[:, jj * 128:(jj + 1) * 128],
                                                idn)
                        nc.vector.tensor_copy(
                            v_n[h][:, c * 4:(c + 1) * 4, 0:64],
                            tp[:].rearrange("p (j d) -> p j d", j=4))

                qkv_chunk(0)
                emit_scores(0, 0, scpool1)
                qkv_chunk(1)
                emit_scores(0, 1, scpool1)
                emit_scores(1, 0, scpool1)
                emit_scores(1, 1, scpool1)
                qkv_chunk(2)
                emit_scores(0, 2, scpool1)
                qkv_chunk(3)

            # ------------- Region 2: steady interleave of scores/PV/proj ------
            with (
                tc.tile_pool(name="scps2", bufs=2, space="PSUM") as scpool2,
                tc.tile_pool(name="acc", bufs=3, space="PSUM") as acpool,
                tc.tile_pool(name="pj", bufs=1, space="PSUM") as pjpool,
            ):
                emit_scores(0, 3, scpool2)
                pv_chunk(0, 0, acpool)
                emit_scores(1, 2, scpool2)
                pv_chunk(0, 1, acpool)
                emit_scores(1, 3, scpool2)
                pv_chunk(0, 2, acpool)
                emit_scores(2, 0, scpool2)
                pv_chunk(0, 3, acpool)
                norm_copy(0)
                emit_scores(2, 1, scpool2)
                norm_fin(0, pjpool)
                pv_chunk(1, 0, acpool)
                emit_scores(2, 2, scpool2)
                proj_piece(0, 0, scpool2)
                pv_chunk(1, 1, acpool)
                emit_scores(2, 3, scpool2)
                proj_piece(0, 1, scpool2)
                pv_chunk(1, 2, acpool)
                emit_scores(3, 0, scpool2)
                proj_piece(0, 2, scpool2)
                pv_chunk(1, 3, acpool)
                norm_copy(1)
                emit_scores(3, 1, scpool2)
                norm_fin(1, pjpool)
                proj_piece(0, 3, scpool2)
                pv_chunk(2, 0, acpool)
                emit_scores(3, 2, scpool2)
                proj_piece(1, 0, scpool2)
                pv_chunk(2, 1, acpool)
                emit_scores(3, 3, scpool2)
                proj_piece(1, 1, scpool2)
                pv_chunk(2, 2, acpool)
                proj_piece(1, 2, scpool2)
                pv_chunk(2, 3, acpool)
                norm_copy(2)
                proj_piece(1, 3, scpool2)
                norm_fin(2, pjpool)
                pv_chunk(3, 0, acpool)
                proj_piece(2, 0, scpool2)
                pv_chunk(3, 1, acpool)
                proj_piece(2, 1, scpool2)
                pv_chunk(3, 2, acpool)
                proj_piece(2, 2, scpool2)
                pv_chunk(3, 3, acpool)
                norm_copy(3)
                proj_piece(2, 3, scpool2)
                norm_fin(3, pjpool)
                for j in range(4):
                    proj_piece(3, j, scpool2, use_act=True)

    nc.compile()
    return nc


def make_in_maps(x, w_qkv, b_qkv, w_proj):
    """Per-core input dicts. Core c: batch c//4, heads 3*(c%4)+[0..2]."""
    x = np.asarray(x, np.float32)
    w_qkv = np.asarray(w_qkv, np.float32)
    b_qkv = np.asarray(b_qkv, np.float32)
    w_proj = np.asarray(w_proj, np.float32)
    q = lambda h: w_qkv[:, h * 64:(h + 1) * 64]
    k = lambda h: w_qkv[:, C + h * 64: C + (h + 1) * 64]
    v = lambda h: w_qkv[:, 2 * C + h * 64: 2 * C + (h + 1) * 64]
    qb = lambda h: b_qkv[h * 64:(h + 1) * 64]
    kb = lambda h: b_qkv[C + h * 64: C + (h + 1) * 64]
    vb = lambda h: b_qkv[2 * C + h * 64: 2 * C + (h + 1) * 64]
    # xt[c, p, t, n] = x[b][c*512+n, t*128+p], bf16
    xt = [np.ascontiguousarray(
        x[b].reshape(NCH, 512, CT, 128).transpose(0, 3, 2, 1)
    ).astype(BF16NP) for b in range(B)]
    in_maps = []
    for c in range(NCORES):
        b = c // 4
        h0 = 3 * (c % 4)
        h1, h2 = h0 + 1, h0 + 2
        w_pack = np.concatenate(
            [q(h0), q(h1), k(h0), k(h1), q(h2), q(h2), k(h2), k(h2),
             v(h0), v(h1), v(h2)], axis=1).astype(np.float32)
        bias = np.concatenate(
            [qb(h0), qb(h1), kb(h0), kb(h1), qb(h2), qb(h2), kb(h2), kb(h2),
             vb(h0), vb(h1), vb(h2), np.zeros(64, np.float32)])
        bq_pack = bias.reshape(6, 128).T.copy()  # [128, 6]
        wp_pack = np.stack(
            [w_proj[h * 64:(h + 1) * 64, :] for h in (h0, h1, h2)])
        im = {
            "xt": xt[b],
            "bq": np.ascontiguousarray(bq_pack),
            "wp": np.ascontiguousarray(wp_pack.transpose(1, 0, 2)),
        }
        off = 0
        for t in range(6):
            mm = TMM[t]
            blk = w_pack[:, off:off + mm]  # [768, mm]
            off += mm
            # w{t}[p, ct, m] = w_pack[ct*128+p, t_off+m]
            im[f"w{t}"] = np.ascontiguousarray(
                blk.reshape(CT, 128, mm).transpose(1, 0, 2)).astype(BF16NP)
        in_maps.append(im)
    return in_maps


_NC_CACHE = []


def _get_program():
    if not _NC_CACHE:
        _NC_CACHE.append(build_program())
    return _NC_CACHE[0]


def run(inputs, trace=False, **kw):
    nc = _get_program()
    in_maps = make_in_maps(inputs["x"], inputs["w_qkv"], inputs["b_qkv"],
                           inputs["w_proj"])
    res = run_bass_kernel_spmd(nc, in_maps, list(range(NCORES)), trace=trace, **kw)
    b_proj = np.asarray(inputs["b_proj"], np.float32)
    out = np.zeros((B, N, C), np.float32)
    for c in range(NCORES):
        out[c // 4] += res.results[c]["y"]
    out += b_proj[None, None, :]
    return out.astype(np.float32), res


def kernel(**inputs):
    out, _ = run(inputs)
    return out
